# revision 1
# baseline (speedup 1.0000x reference)
"""KANLinear TRN2 Bass kernel (8-core SPMD, token-data-parallel).

Math (matches the jax reference exactly, up to fp rounding):
  y[b,o] = silu(x)[b,:] @ scale_base.T  +  sum_{i,g} B_g(x[b,i]) * w[o,i,g]
with cubic B-spline bases on the uniform grid t_j = -1.75 + 0.25*j
(j = 0..14, 11 bases). On-device identity (truncated-power form): with
  xh  = clamp(x, -1.75, 1.75)
  c_j = relu(4*xh + 7 - j)^3
the basis is the exact 4th difference
  6*B_g(x) = c_g - 4*c_{g+1} + 6*c_{g+2} - 4*c_{g+3} + c_{g+4}.
The x-clamp keeps |arguments| <= 14 (bounds fp32 cancellation error to
~3e-4 absolute on B) and reproduces the reference's all-zero basis rows
outside the grid exactly. The 1/6 is folded into the host-prepared
weights.

Sharding: tokens (8192) split 1024/core across 8 cores; grid/coeff/
scale_base replicated (coeff pre-transposed on host to the matmul
K-order k = it*1408 + g*128 + p, i.e. [it, g, p, o]).

Per core the main einsum is a [1024 x 11264] @ [11264 x 1024] matmul
in bf16 (fp32 PSUM accumulation), fed by on-device computed basis
tiles; the silu base matmul accumulates into the same PSUM banks.
"""

import numpy as np
import ml_dtypes

import concourse.bass as bass
import concourse.mybir as mybir
import concourse.tile as tile
from concourse import bacc
from concourse.alu_op_type import AluOpType
from concourse.bass_utils import run_bass_kernel_spmd

AF = mybir.ActivationFunctionType
F32 = mybir.dt.float32
BF16 = mybir.dt.bfloat16

# problem constants (hardcoded per the task contract)
TOKENS, IN_DIM, OUT_DIM = 8192, 1024, 1024
GRID_SIZE, K = 8, 3
NCHAN = GRID_SIZE + 2 * K + 1  # 15 truncated-power channels
NBASIS = GRID_SIZE + K  # 11 bases
N_CORES = 8
TPC = TOKENS // N_CORES  # tokens per core (1024)
HALF = 512  # tokens per processing chunk (PSUM-bank limited)
NIT = IN_DIM // 128  # in-dim tiles (8)
M_TILES = HALF // 128  # token tiles per half (4)
N_OC = OUT_DIM // 512  # out-dim chunks (2)

X_CLAMP = 1.75
COMB_W = NBASIS * HALF  # 5632: combine ops span all 11 bases at once

_CACHED = None


def _build_bass():
    nc = bacc.Bacc("TRN2", target_bir_lowering=False, debug=False,
                   num_devices=N_CORES)
    xt = nc.declare_dram_parameter("xt", [IN_DIM, TPC], F32, isOutput=False)
    w2 = nc.declare_dram_parameter("w2", [NBASIS * IN_DIM, OUT_DIM], BF16,
                                   isOutput=False)
    sbt = nc.declare_dram_parameter("sbt", [IN_DIM, OUT_DIM], BF16,
                                    isOutput=False)
    y = nc.declare_dram_parameter("y", [TPC, OUT_DIM], F32, isOutput=True)

    with tile.TileContext(nc) as tc:
        with (
            tc.tile_pool(name="xts", bufs=4) as xpool,
            tc.tile_pool(name="silu", bufs=9) as spool,
            tc.tile_pool(name="cbuf", bufs=2) as cpool,
            tc.tile_pool(name="sq", bufs=2) as sqpool,
            tc.tile_pool(name="ctmp", bufs=1) as tpool,
            tc.tile_pool(name="bbuf", bufs=3) as bpool,
            tc.tile_pool(name="wts", bufs=4) as wpool,
            tc.tile_pool(name="outs", bufs=4) as opool,
            tc.tile_pool(name="consts", bufs=1) as kpool,
            tc.tile_pool(name="psum", bufs=8, space="PSUM") as ppool,
        ):
            bias_tile = kpool.tile([128, NCHAN + 1], F32, tag="bias")
            for j in range(NCHAN):
                # q_j = relu(-4*r + (14-j)); r = relu(1.75 - x)
                nc.vector.memset(bias_tile[:, j:j + 1], float(14 - j))
            nc.vector.memset(bias_tile[:, NCHAN:NCHAN + 1], X_CLAMP)
            for half in range(2):
                t0 = half * HALF
                # x^T tiles for this half + silu(x)^T (bf16) up-front so the
                # ACT table set for Silu is used in one batch
                xt_tiles = []
                silu_tiles = []
                for it in range(NIT):
                    xtt = xpool.tile([128, HALF], F32, tag="xt")
                    nc.sync.dma_start(out=xtt,
                                      in_=xt[it * 128:(it + 1) * 128,
                                             t0:t0 + HALF])
                    xt_tiles.append(xtt)
                r_tiles = []
                for it in range(NIT):
                    st = spool.tile([128, HALF], BF16, tag="silu")
                    nc.scalar.activation(st, xt_tiles[it], AF.Silu)
                    silu_tiles.append(st)
                    # one-sided clamp via ACT: r = relu(1.75 - x), so that
                    # relu(-4r + (14-j)) == relu(4*min(x,1.75) + 7 - j)
                    rt = spool.tile([128, HALF], F32, tag="rt")
                    nc.scalar.activation(rt, xt_tiles[it], AF.Relu,
                                         bias=bias_tile[:, NCHAN:NCHAN + 1],
                                         scale=-1.0)
                    r_tiles.append(rt)

                psums = [[ppool.tile([128, 512], F32, tag="ps",
                                     name=f"ps_{half}_{_oc}_{_m}")
                          for _m in range(M_TILES)] for _oc in range(N_OC)]

                kt_idx = 0
                n_kt = NIT * NBASIS + NIT
                for it in range(NIT):
                    # 14 channels (channel 14 is identically 0 since
                    # relu(4*min(x,1.75)-7) == 0), contiguous in the free
                    # dim of one tile:
                    # q_j = relu(-4*r + (14-j)) = relu(4*min(x,1.75) + 7 - j)
                    NCH = NCHAN - 1  # 14
                    c = cpool.tile([128, NCH * HALF], F32, tag="c")
                    for j in range(NCH):
                        nc.scalar.activation(
                            c[:, j * HALF:(j + 1) * HALF], r_tiles[it],
                            AF.Relu, bias=bias_tile[:, j:j + 1], scale=-4.0)
                    # cube: ACT squares (two big-FD ops on ACT's own SBUF
                    # port), one big DVE multiply. GPSIMD is kept idle: any
                    # Pool op takes an exclusive lock on the SBUF port pair
                    # that DVE's 2-input ops need, fully blocking them.
                    HW = NCH * HALF // 2
                    sqs = []
                    for piece in range(2):
                        csl = c[:, piece * HW:(piece + 1) * HW]
                        sq = sqpool.tile([128, HW], F32, tag="sq")
                        nc.scalar.activation(sq, csl, AF.Square)
                        sqs.append(sq)
                    for piece in range(2):
                        csl = c[:, piece * HW:(piece + 1) * HW]
                        nc.vector.tensor_mul(csl, sqs[piece], csl)
                    # 4th-difference combine into 11 basis tiles (bf16), DVE
                    b = bpool.tile([128, COMB_W], BF16, tag="B")
                    tmp = tpool.tile([128, COMB_W], F32, tag="tA")
                    W10 = (NBASIS - 1) * HALF  # 5120: bases 0..9
                    sl = lambda j, w=COMB_W: c[:, j * HALF:j * HALF + w]
                    nc.vector.scalar_tensor_tensor(tmp, sl(1), -4.0, sl(0),
                                                   AluOpType.mult,
                                                   AluOpType.add)
                    nc.vector.scalar_tensor_tensor(tmp, sl(2), 6.0, tmp,
                                                   AluOpType.mult,
                                                   AluOpType.add)
                    # basis 10 has no +c_14 term: finish it here (bf16 out)
                    nc.vector.scalar_tensor_tensor(
                        b[:, W10:], sl(3, COMB_W)[:, W10:], -4.0,
                        tmp[:, W10:], AluOpType.mult, AluOpType.add)
                    nc.vector.scalar_tensor_tensor(tmp[:, :W10],
                                                   sl(3, W10), -4.0,
                                                   tmp[:, :W10],
                                                   AluOpType.mult,
                                                   AluOpType.add)
                    nc.vector.tensor_add(b[:, :W10], tmp[:, :W10], sl(4, W10))
                    # 11 K-tiles of the spline matmul
                    for g in range(NBASIS):
                        w2t = wpool.tile([128, OUT_DIM], BF16, tag="w2")
                        row = (it * NBASIS + g) * 128
                        nc.sync.dma_start(out=w2t, in_=w2[row:row + 128, :])
                        for oc in range(N_OC):
                            for m in range(M_TILES):
                                nc.tensor.matmul(
                                    psums[oc][m],
                                    lhsT=b[:, g * HALF + m * 128:
                                           g * HALF + m * 128 + 128],
                                    rhs=w2t[:, oc * 512:(oc + 1) * 512],
                                    start=(kt_idx == 0),
                                    stop=(kt_idx == n_kt - 1))
                        kt_idx += 1
                # silu base matmul, accumulated into the same banks
                for it in range(NIT):
                    sbtt = wpool.tile([128, OUT_DIM], BF16, tag="sbt")
                    nc.sync.dma_start(out=sbtt,
                                      in_=sbt[it * 128:(it + 1) * 128, :])
                    for oc in range(N_OC):
                        for m in range(M_TILES):
                            nc.tensor.matmul(
                                psums[oc][m],
                                lhsT=silu_tiles[it][:, m * 128:m * 128 + 128],
                                rhs=sbtt[:, oc * 512:(oc + 1) * 512],
                                start=(kt_idx == 0),
                                stop=(kt_idx == n_kt - 1))
                    kt_idx += 1
                # drain PSUM -> SBUF -> HBM
                for oc in range(N_OC):
                    for m in range(M_TILES):
                        ot = opool.tile([128, 512], F32, tag="out")
                        nc.scalar.copy(ot, psums[oc][m])
                        r0 = t0 + m * 128
                        nc.sync.dma_start(
                            out=y[r0:r0 + 128, oc * 512:(oc + 1) * 512],
                            in_=ot)
    nc.compile()
    return nc


def _prepare_inputs(x, coeff, scale_base, scale_spline):
    x = np.asarray(x, dtype=np.float32)
    coeff = np.asarray(coeff, dtype=np.float32)
    scale_base = np.asarray(scale_base, dtype=np.float32)
    ss = float(np.asarray(scale_spline).reshape(-1)[0])
    # K-order: k = it*1408 + g*128 + p  ->  w2[k, o] = coeff[o, it*128+p, g]
    # (1/6 from the 4th-difference identity folded in here)
    w2 = (coeff * (ss / 6.0)).reshape(OUT_DIM, NIT, 128, NBASIS)
    w2 = np.ascontiguousarray(w2.transpose(1, 3, 2, 0)).reshape(
        NBASIS * IN_DIM, OUT_DIM)
    w2 = w2.astype(ml_dtypes.bfloat16)
    sbt = np.ascontiguousarray(scale_base.T).astype(ml_dtypes.bfloat16)
    in_maps = []
    for c in range(N_CORES):
        xt = np.ascontiguousarray(x[c * TPC:(c + 1) * TPC, :].T)
        in_maps.append({"xt": xt, "w2": w2, "sbt": sbt})
    return in_maps


def _get_bass():
    global _CACHED
    if _CACHED is None:
        _CACHED = _build_bass()
    return _CACHED


def run(inputs, trace=False, **kw):
    nc = _get_bass()
    in_maps = _prepare_inputs(inputs["x"], inputs["coeff"],
                              inputs["scale_base"], inputs["scale_spline"])
    res = run_bass_kernel_spmd(nc, in_maps, list(range(N_CORES)),
                               trace=trace, **kw)
    y = np.concatenate([np.asarray(res.results[c]["y"])
                        for c in range(N_CORES)], axis=0)
    return np.ascontiguousarray(y.astype(np.float32)), res


def kernel(x, grid, coeff, scale_base, scale_spline):
    y, _ = run({"x": x, "grid": grid, "coeff": coeff,
                "scale_base": scale_base, "scale_spline": scale_spline})
    return y

